# revision 2
# baseline (speedup 1.0000x reference)
"""GCN (2-layer + pvt projection) Trainium2 kernel v2, 8-core SPMD.

Node (destination) parallelism across 8 cores, n_loc = 12800 rows/core.
Node table layout uses sub-block quarters: quarter j = concat over cores of
each core's j-th sub-block (25 tiles), so each table AllGather splits into 4
pipelined sub-AllGathers and gather indices stay int16 (< 25600).

Per SpMM pass: edges sorted by (group, src-quarter, dest); slots padded to
128 only per (group, quarter) => gathers are one dma_gather per (g,q) piece.
Chunks of 128 slots may straddle tiles; each (chunk, tile) piece is one
matmul accumulating into the tile's feature-major PSUM [nf, 128] at free
offset [lo, lo+M). A zero matmul (start=True) covers the full span first.
Bands (val at (slot, destpos)) are fp8 (adj vals x16, consume scales 1/16),
resident in SBUF for the two adj passes.
"""

import sys

sys.path.insert(0, "/opt/trn_rl_repo")

import numpy as np
import ml_dtypes

from concourse import bass, bacc, mybir, tile
from concourse import bass_utils
from concourse.bass_utils import run_bass_kernel_spmd

# ---- NTFF profiling hook (normally injected by the launcher) -------------


def _install_ntff_hook():
    import types
    import ctypes
    import contextlib

    if "antenv.axon_hooks" in sys.modules:
        return
    hook = None
    so_path = "/opt/axon/libaxon_pjrt.so"
    try:
        lib = ctypes.CDLL(so_path)
        if hasattr(lib, "axon_start_nrt_profile"):
            lib.axon_start_nrt_profile.argtypes = [
                ctypes.POINTER(ctypes.c_int64), ctypes.c_size_t]
            lib.axon_start_nrt_profile.restype = ctypes.c_int64
            lib.axon_stop_nrt_profile.argtypes = [ctypes.c_char_p]
            lib.axon_stop_nrt_profile.restype = ctypes.c_int64

            @contextlib.contextmanager
            def _hook(output_dir, device_ids):
                import jax
                jax.devices()
                if device_ids:
                    ids = (ctypes.c_int64 * len(device_ids))(*device_ids)
                    rc = lib.axon_start_nrt_profile(ids, len(device_ids))
                else:
                    rc = lib.axon_start_nrt_profile(None, 0)
                if rc != 0:
                    raise RuntimeError(f"axon_start_nrt_profile rc={rc}")
                try:
                    yield
                finally:
                    n = lib.axon_stop_nrt_profile(str(output_dir).encode())
                    print(f"ntff profile: {n} file(s) -> {output_dir}")

            hook = _hook
    except OSError:
        pass
    mod = types.ModuleType("antenv.axon_hooks")
    mod.get_axon_ntff_profile_hook = lambda: hook
    mod.set_axon_ntff_profile_hook = lambda h: None
    sys.modules["antenv.axon_hooks"] = mod


_install_ntff_hook()
bass_utils.upload_artifacts = lambda tmpdir: f"local://{tmpdir}"

BF16 = ml_dtypes.bfloat16
FP8 = ml_dtypes.float8_e4m3fn
NCORES = 8
NQ = 4
P = 128
GMAX = 3968  # max idxs per dma_gather (ring-safe: 126 of 128 entries)

FULL = dict(N=100_000, NFEAT=512, NHID=64, NCLASS=40)


# --------------------------------------------------------------------------
# host-side planning
# --------------------------------------------------------------------------

class Plan2:
    """Straddle-chunk SpMM plan. Core-uniform structure, per-core tensors."""

    def __init__(self, rows, cols, vals, n_loc, G_T, val_scale,
                 per_tile_pad=False, full_span=False):
        n_tiles = n_loc // P
        assert n_tiles % NQ == 0 and n_tiles % G_T == 0
        sb_t = n_tiles // NQ          # tiles per sub-block
        sb_r = sb_t * P               # rows per sub-block
        qrows = sb_r * NCORES         # rows per table quarter
        assert qrows <= 32767
        self.n_tiles, self.qrows, self.G_T = n_tiles, qrows, G_T
        groups = [list(range(s, s + G_T)) for s in range(0, n_tiles, G_T)]
        self.groups = groups
        NG = len(groups)

        # ---- map sources to (quarter, qpos) ----
        k_src = cols // n_loc
        r_src = cols % n_loc
        j_src = r_src // sb_r
        qpos = k_src * sb_r + (r_src % sb_r)

        # ---- shard by dest core, sort by (group, quarter, dest) ----
        # padding bucket: (t, q) when per_tile_pad else (g, q)
        core = rows // n_loc
        per = []
        NB = n_tiles if per_tile_pad else NG
        counts = np.zeros((NCORES, NB, NQ), np.int64)
        for k in range(NCORES):
            m = core == k
            d = (rows[m] - k * n_loc).astype(np.int64)
            q = j_src[m].astype(np.int64)
            qp = qpos[m].astype(np.int64)
            v = vals[m].astype(np.float32)
            g = d // (P * G_T)
            b = (d // P) if per_tile_pad else g
            o = np.lexsort((d, q, g))
            g, q, b, d, qp, v = g[o], q[o], b[o], d[o], qp[o], v[o]
            np.add.at(counts[k], (b, q), 1)
            per.append((b, q, d, qp, v))

        # ---- bucket slot spans: padded to 128, uniform max across cores ----
        bspan = -(-counts.max(axis=0) // P) * P  # [NB, NQ]
        b_off = np.zeros((NB, NQ), np.int64)
        gq_off = np.zeros((NG, NQ), np.int64)
        span = np.zeros((NG, NQ), np.int64)
        off = 0
        tpb = n_tiles // NB  # 1 if per-tile buckets else G_T
        for g in range(NG):
            for q in range(NQ):
                gq_off[g, q] = off
                if per_tile_pad:
                    for t in groups[g]:
                        b_off[t, q] = off
                        off += bspan[t, q]
                else:
                    b_off[g, q] = off
                    off += bspan[g, q]
                span[g, q] = off - gq_off[g, q]
        S = off
        self.S = S
        self.gq_off = gq_off
        self.span = span
        cid_of_slot = np.arange(S) // P

        # ---- per-core slot-aligned arrays ----
        slot_qp = np.zeros((NCORES, S), np.int64)
        slot_val = np.zeros((NCORES, S), np.float32)
        slot_d = np.full((NCORES, S), -1, np.int64)
        for k in range(NCORES):
            b, q, d, qp, v = per[k]
            key = b * NQ + q
            ne = len(key)
            if ne:
                starts = np.r_[0, np.nonzero(np.diff(key))[0] + 1]
                run_id = np.zeros(ne, np.int64)
                run_id[starts[1:]] = 1
                run_id = np.cumsum(run_id)
                rank = np.arange(ne) - starts[run_id]
                s = b_off[b, q] + rank
                slot_qp[k, s] = qp
                slot_val[k, s] = v
                slot_d[k, s] = d

        # ---- chunk pieces: per (chunk, tile) union [lo, hi) across cores --
        lo_ct = {}
        hi_ct = {}
        for k in range(NCORES):
            sl = np.nonzero(slot_d[k] >= 0)[0]
            ci = cid_of_slot[sl]
            t = slot_d[k, sl] // P
            pos = slot_d[k, sl] % P
            key = ci * n_tiles + t
            uk, inv = np.unique(key, return_inverse=True)
            mn = np.full(len(uk), P, np.int64)
            mx = np.zeros(len(uk), np.int64)
            np.minimum.at(mn, inv, pos)
            np.maximum.at(mx, inv, pos + 1)
            for i, kk in enumerate(uk):
                cc, tt = divmod(int(kk), n_tiles)
                cur = lo_ct.get((cc, tt))
                if cur is None:
                    lo_ct[(cc, tt)] = int(mn[i])
                    hi_ct[(cc, tt)] = int(mx[i])
                else:
                    lo_ct[(cc, tt)] = min(cur, int(mn[i]))
                    hi_ct[(cc, tt)] = max(hi_ct[(cc, tt)], int(mx[i]))

        # ---- emission-ordered pieces per (g, q) with first/last flags ----
        # first piece of each tile is widened to [0, 128) (PSUM start).
        order = []  # (g, q, ci, t)
        for g in range(NG):
            for q in range(NQ):
                b0 = gq_off[g, q]
                for c in range(span[g, q] // P):
                    ci = (b0 + c * P) // P
                    for t in groups[g]:
                        if (ci, t) in lo_ct:
                            order.append((g, q, int(ci), t))
        seen = set()
        npieces = {}
        for (g, q, ci, t) in order:
            if t not in seen or full_span:
                seen.add(t)
                lo_ct[(ci, t)] = 0
                hi_ct[(ci, t)] = P
            npieces[t] = npieces.get(t, 0) + 1
        assert len(seen) == n_tiles, f"tiles without pieces: {n_tiles-len(seen)}"
        self.pieces = {(g, q): [] for g in range(NG) for q in range(NQ)}
        piece_off = {}
        lhs_off = 0
        seen = set()
        ate = {}
        for (g, q, ci, t) in order:
            lo = lo_ct[(ci, t)]
            M = hi_ct[(ci, t)] - lo
            first = t not in seen
            seen.add(t)
            ate[t] = ate.get(t, 0) + 1
            last = ate[t] == npieces[t]
            cl = ci - gq_off[g, q] // P
            self.pieces[(g, q)].append(
                (int(cl), t, lhs_off, int(M), int(lo), first, last))
            piece_off[(ci, t)] = lhs_off
            lhs_off += M
        self.L = lhs_off

        # ---- per-core band (fp8) + idx tensors ----
        self.band_np = []
        self.idx_np = []
        for k in range(NCORES):
            sl = np.nonzero(slot_d[k] >= 0)[0]
            ci = cid_of_slot[sl]
            t = slot_d[k, sl] // P
            pos = slot_d[k, sl] % P
            band = np.zeros((P, self.L), np.float32)
            coli = np.array([piece_off[(int(c), int(tt))] for c, tt in
                             zip(ci, t)], np.int64)
            coli += pos - np.array([lo_ct[(int(c), int(tt))] for c, tt in
                                    zip(ci, t)], np.int64)
            band[sl % P, coli] = slot_val[k, sl] * val_scale
            self.band_np.append(band.astype(FP8))
            idx16 = np.zeros((16, S // 16), np.int16)
            ss = np.arange(S)
            idx16[ss % 16, ss // 16] = slot_qp[k].astype(np.int16)
            self.idx_np.append(np.tile(idx16, (NCORES, 1)))

        # ---- gather call list per (g, q): (slot_off, n_idx) ----
        self.gathers = {}
        cmax = 0
        for g in range(NG):
            for q in range(NQ):
                n = int(span[g, q])
                b0 = int(gq_off[g, q])
                cmax = max(cmax, n // P)
                calls = []
                while n > 0:
                    take = min(n, GMAX)
                    take -= take % P
                    calls.append((b0, take))
                    b0 += take
                    n -= take
                self.gathers[(g, q)] = calls
        self.cmax = cmax


# --------------------------------------------------------------------------
# device kernel builder
# --------------------------------------------------------------------------

def build_kernel2(dims, ep, pp):
    n_loc = dims["n_loc"]
    NFEAT, NHID, NCLASS = dims["NFEAT"], dims["NHID"], dims["NCLASS"]
    n_tiles = n_loc // P
    ncc = NFEAT // P
    sb_t = n_tiles // NQ
    sb_r = sb_t * P
    qrows = sb_r * NCORES
    n_pad = n_loc * NCORES
    f32 = mybir.dt.float32
    bf16 = mybir.dt.bfloat16
    fp8 = mybir.dt.float8e4
    i16 = mybir.dt.int16

    nc = bacc.Bacc("TRN2", target_bir_lowering=False, debug=False,
                   enable_asserts=False, num_devices=NCORES,
                   num_swdge_queues=4)

    xT_d = nc.dram_tensor("xT", [NFEAT, n_loc], bf16, kind="ExternalInput")
    w1_d = nc.dram_tensor("w1", [NFEAT, NHID], bf16, kind="ExternalInput")
    w2_d = nc.dram_tensor("w2", [NHID, NCLASS], bf16, kind="ExternalInput")
    b1_d = nc.dram_tensor("b1", [NHID, 1], f32, kind="ExternalInput")
    b2_d = nc.dram_tensor("b2c", [NCLASS, 1], f32, kind="ExternalInput")
    eidx_d = nc.dram_tensor("eidx", [P, ep.S // 16], i16, kind="ExternalInput")
    ebnd_d = nc.dram_tensor("ebnd", [P, ep.L], fp8, kind="ExternalInput")
    pidx_d = nc.dram_tensor("pidx", [P, pp.S // 16], i16, kind="ExternalInput")
    pbnd_d = nc.dram_tensor("pbnd", [P, pp.L], fp8, kind="ExternalInput")
    out_d = nc.dram_tensor("out", [n_loc, NCLASS], f32, kind="ExternalOutput")

    rg = [list(range(NCORES))]

    with tile.TileContext(nc) as tc:
        with (
            tc.tile_pool(name="dram", bufs=1, space="DRAM") as dram,
            tc.tile_pool(name="const", bufs=1) as cpool,
            tc.tile_pool(name="xt", bufs=2) as xtp,
            tc.tile_pool(name="fbuf", bufs=12) as fpool,
            tc.tile_pool(name="bnd", bufs=2) as bpool,
            tc.tile_pool(name="stg", bufs=4) as spool,
            tc.tile_pool(name="psum", bufs=6, space="PSUM") as pspool,
            tc.tile_pool(name="psumt", bufs=2, space="PSUM") as ptpool,
        ):
            ag1_in = dram.tile([n_loc, P], bf16, tag="ag1_in")
            ag2_in = dram.tile([n_loc, P], bf16, tag="ag2_in")
            ag3_in = dram.tile([n_loc, P], bf16, tag="ag3_in")
            tabs = {}
            for nm in ("tab1", "tab2", "tab3"):
                for j in range(NQ):
                    tq = dram.tile([qrows, P], bf16, tag=f"{nm}q{j}",
                                   name=f"{nm}q{j}", addr_space="Shared")
                    tabs[(nm, j)] = tq
            tab1 = [tabs[("tab1", j)] for j in range(NQ)]
            tab2 = [tabs[("tab2", j)] for j in range(NQ)]
            tab3 = [tabs[("tab3", j)] for j in range(NQ)]

            # ---- constants / resident tensors ----
            w1_sb = cpool.tile([P, ncc, NHID], bf16, tag="w1")
            nc.sync.dma_start(
                out=w1_sb[:],
                in_=w1_d.ap().rearrange("(c p) f -> p c f", p=P))
            w2_sb = cpool.tile([NHID, NCLASS], bf16, tag="w2")
            nc.sync.dma_start(out=w2_sb[:], in_=w2_d.ap())
            b1_sb = cpool.tile([NHID, 1], f32, tag="b1")
            nc.sync.dma_start(out=b1_sb[:], in_=b1_d.ap())
            b2_sb = cpool.tile([NCLASS, 1], f32, tag="b2")
            nc.sync.dma_start(out=b2_sb[:], in_=b2_d.ap())
            ident = cpool.tile([P, P], bf16, tag="ident")
            from concourse.masks import make_identity
            make_identity(nc, ident[:])
            zband = cpool.tile([P, P], bf16, tag="zband")
            nc.vector.memset(zband[:], 0.0)

            eidx_sb = cpool.tile([P, ep.S // 16], i16, tag="eidx")
            nc.sync.dma_start(out=eidx_sb[:], in_=eidx_d.ap())
            pidx_sb = cpool.tile([P, pp.S // 16], i16, tag="pidx")
            nc.sync.dma_start(out=pidx_sb[:], in_=pidx_d.ap())

            h1T = cpool.tile([NHID, n_loc], bf16, tag="h1T")

            def emit_table_tile(ag_in, t, src_ap, nf):
                stg = spool.tile([P, P], bf16, tag="stg")
                nc.vector.memset(stg[:, nf:P], 0.0)
                nc.vector.tensor_copy(out=stg[:, 0:nf], in_=src_ap)
                nc.sync.dma_start(out=ag_in[t * P:(t + 1) * P, :], in_=stg[:])

            def sub_allgather(ag_in, tab, j):
                nc.gpsimd.collective_compute(
                    "AllGather", mybir.AluOpType.bypass, replica_groups=rg,
                    ins=[ag_in[j * sb_r:(j + 1) * sb_r, :].opt()],
                    outs=[tab[j][:, :].opt()])

            # tiny warmup collective to absorb the CC-stream init barrier
            wrm_in = dram.tile([8, P], bf16, tag="wrm_in")
            wrm_out = dram.tile([64, P], bf16, tag="wrm_out",
                                addr_space="Shared")
            wrm_sb = spool.tile([8, P], bf16, tag="wrm")
            nc.vector.memset(wrm_sb[:], 0.0)
            nc.sync.dma_start(out=wrm_in[:, :], in_=wrm_sb[:])
            nc.gpsimd.collective_compute(
                "AllGather", mybir.AluOpType.bypass, replica_groups=rg,
                ins=[wrm_in[:, :].opt()], outs=[wrm_out[:, :].opt()])

            # ================= stage A: XW1 + AG1 (per sub-block) =========
            for XB in (5, 4, 3, 2, 1):  # XW1 tile batch (divides sb_t)
                if sb_t % XB == 0:
                    break
            for j in range(NQ):
                for t0 in range(j * sb_t, (j + 1) * sb_t, XB):
                    tl = list(range(t0, t0 + XB))
                    r0 = t0 * P
                    nrows = XB * P
                    xt = xtp.tile([P, ncc, XB * P], bf16, tag="xt")
                    nc.sync.dma_start(
                        out=xt[:, :, 0:nrows],
                        in_=xT_d.ap().rearrange(
                            "(c p) r -> p c r", p=P)[:, :, r0:r0 + nrows])
                    for t in tl:
                        ps = pspool.tile([P, NHID], f32, tag="ps")
                        for cc in range(ncc):
                            nc.tensor.matmul(
                                out=ps[:],
                                lhsT=xt[:, cc, t * P - r0:(t + 1) * P - r0],
                                rhs=w1_sb[:, cc, :],
                                start=(cc == 0), stop=(cc == ncc - 1))
                        emit_table_tile(ag1_in, t, ps[:], NHID)
                sub_allgather(ag1_in, tab1, j)

            # ================= generic SpMM pass (mode T) =================
            def spmm(plan, tab, idx_sb, band_d, band_tag, nf, consume,
                     mode="T"):
                for g, tl in enumerate(plan.groups):
                    fbs = {}
                    for q in range(NQ):
                        fb = fpool.tile([P, plan.cmax, P], bf16, tag="F")
                        b0 = plan.gq_off[g, q]
                        for (soff, n) in plan.gathers[(g, q)]:
                            c0 = (soff - b0) // P
                            nc.gpsimd.dma_gather(
                                fb[:, c0:c0 + n // P, :],
                                tab[q][:, :],
                                idx_sb[:, soff // 16:(soff + n) // 16],
                                n, n, P, elem_step=P, single_packet=False,
                                queue_num=q)
                        fbs[q] = fb
                    g0 = plan.group_band_span[g]
                    g1 = (plan.group_band_span[g + 1]
                          if g + 1 < len(plan.groups) else plan.L)
                    bnd = bpool.tile([P, plan.pb_max], fp8, tag=band_tag,
                                     bufs=3 if band_tag == "eb" else 2)
                    if g1 > g0:
                        nc.sync.dma_start(out=bnd[:, 0:g1 - g0],
                                          in_=band_d.ap()[:, g0:g1])
                    pss = {}
                    prev = {}
                    for t in tl:
                        ps = pspool.tile([nf, P] if mode == "T" else [P, nf],
                                         f32, tag="ps")
                        pss[t] = ps
                    for q in range(NQ):
                        for (cl, t, loff, M, lo, first, last) in \
                                plan.pieces[(g, q)]:
                            if mode == "T":
                                mm = nc.tensor.matmul(
                                    out=pss[t][:, lo:lo + M],
                                    lhsT=fbs[q][:, cl, 0:nf],
                                    rhs=bnd[:, loff - g0:loff - g0 + M],
                                    start=first, stop=last)
                            else:
                                assert lo == 0 and M == P
                                mm = nc.tensor.matmul(
                                    out=pss[t][:],
                                    lhsT=bnd[:, loff - g0:loff - g0 + M],
                                    rhs=fbs[q][:, cl, 0:nf],
                                    start=first, stop=last)
                            if t in prev:
                                tile.add_dep_helper(
                                    mm.ins, prev[t].ins, sync=False,
                                    reason="acc order")
                            prev[t] = mm
                    for t in tl:
                        consume(t, pss[t])

            # ================= gc1 + B2 + AG2 =============================
            def gc1_consume(t, ps):
                nc.scalar.activation(
                    out=h1T[:, t * P:(t + 1) * P], in_=ps[:],
                    func=mybir.ActivationFunctionType.Relu,
                    bias=b1_sb[:, 0:1], scale=1.0 / 16.0)
                ps2 = pspool.tile([P, NCLASS], f32, tag="ps")
                nc.tensor.matmul(out=ps2[:],
                                 lhsT=h1T[:, t * P:(t + 1) * P],
                                 rhs=w2_sb[:], start=True, stop=True)
                emit_table_tile(ag2_in, t, ps2[:], NCLASS)
                if (t + 1) % sb_t == 0:
                    sub_allgather(ag2_in, tab2, t // sb_t)

            spmm(ep, tab1, eidx_sb, ebnd_d, "eb", NHID, gc1_consume)

            # ================= gc2 + AG3 ==================================
            def gc2_consume(t, ps):
                h2t = spool.tile([NHID, P], bf16, tag="h2t")
                nc.vector.memset(h2t[32:NHID, :], 0.0)
                nc.vector.tensor_scalar(
                    out=h2t[0:NCLASS, :], in0=ps[:],
                    scalar1=1.0 / 16.0, scalar2=b2_sb[:, 0:1],
                    op0=mybir.AluOpType.mult, op1=mybir.AluOpType.add)
                pst = ptpool.tile([P, NHID], bf16, tag="pst")
                nc.tensor.transpose(out=pst[:], in_=h2t[:],
                                    identity=ident[0:NHID, 0:NHID])
                emit_table_tile(ag3_in, t, pst[:], NHID)
                if (t + 1) % sb_t == 0:
                    sub_allgather(ag3_in, tab3, t // sb_t)

            spmm(ep, tab2, eidx_sb, ebnd_d, "eb", NCLASS, gc2_consume)

            # ===== pvt (mode A) + log_softmax (batched ln, no max-sub) ====
            # logits are tiny (|z| < 1), so exp without max-subtraction is
            # safe in f32; one Ln at the end avoids per-tile ACT table swaps.
            zb_all = cpool.tile([P, n_tiles, NCLASS], bf16, tag="zb_all")
            st_all = cpool.tile([P, n_tiles], f32, tag="st_all")

            def pvt_consume(t, ps):
                nc.vector.tensor_copy(out=zb_all[:, t, :], in_=ps[:])
                eb = spool.tile([P, NCLASS], f32, tag="eb")
                nc.scalar.activation(out=eb[:], in_=ps[:],
                                     func=mybir.ActivationFunctionType.Exp,
                                     accum_out=st_all[:, t:t + 1])

            spmm(pp, tab3, pidx_sb, pbnd_d, "pb", NCLASS, pvt_consume,
                 mode="A")
            lst_all = cpool.tile([P, n_tiles], f32, tag="lst_all")
            nc.scalar.activation(out=lst_all[:], in_=st_all[:],
                                 func=mybir.ActivationFunctionType.Ln)
            for t in range(n_tiles):
                ob = spool.tile([P, NCLASS], f32, tag="ob")
                nc.vector.tensor_scalar(
                    out=ob[:], in0=zb_all[:, t, :],
                    scalar1=lst_all[:, t:t + 1], scalar2=None,
                    op0=mybir.AluOpType.subtract)
                nc.sync.dma_start(out=out_d.ap()[t * P:(t + 1) * P, :],
                                  in_=ob[:])

    nc.compile()
    return nc


# --------------------------------------------------------------------------
# host wrapper
# --------------------------------------------------------------------------

def _plan_all(inputs, dims, G_Te=4, G_Tp=5):
    N = dims["N"]
    n_loc = dims["n_loc"]
    ep = Plan2(inputs["adj_row"].astype(np.int64),
               inputs["adj_col"].astype(np.int64),
               np.asarray(inputs["adj_val"], np.float32),
               n_loc, G_Te, 16.0)
    pp = Plan2(inputs["pvt_row"].astype(np.int64),
               inputs["pvt_col"].astype(np.int64),
               np.asarray(inputs["pvt_val"], np.float32),
               n_loc, G_Tp, 1.0, per_tile_pad=True, full_span=True)
    for pl in (ep, pp):
        NGp = len(pl.groups)
        spans = []
        for g in range(NGp):
            offs = [pc[2] for q in range(NQ) for pc in pl.pieces[(g, q)]]
            spans.append(min(offs) if offs else (spans[-1] if spans else 0))
        pl.group_band_span = spans
        pb_max = 0
        for g in range(NGp):
            g0 = spans[g]
            g1 = spans[g + 1] if g + 1 < NGp else pl.L
            pb_max = max(pb_max, g1 - g0)
        pl.pb_max = max(pb_max, 1)
    return ep, pp


def _prep_inputs(inputs, dims, ep, pp):
    N, NFEAT = dims["N"], dims["NFEAT"]
    NHID, NCLASS = dims["NHID"], dims["NCLASS"]
    n_loc = dims["n_loc"]
    n_pad = n_loc * NCORES
    x_pad = np.zeros((n_pad, NFEAT), np.float32)
    x_pad[:N] = np.asarray(inputs["x"], np.float32)
    w1 = np.asarray(inputs["W1"], np.float32).astype(BF16)
    w2 = np.asarray(inputs["W2"], np.float32).astype(BF16)
    b1 = np.asarray(inputs["b1"], np.float32).reshape(NHID, 1)
    b2c = np.asarray(inputs["b2"], np.float32).reshape(NCLASS, 1).copy()
    in_maps = []
    for k in range(NCORES):
        xT = np.ascontiguousarray(
            x_pad[k * n_loc:(k + 1) * n_loc].T).astype(BF16)
        in_maps.append({
            "xT": xT, "w1": w1, "w2": w2, "b1": b1, "b2c": b2c,
            "eidx": ep.idx_np[k], "ebnd": ep.band_np[k],
            "pidx": pp.idx_np[k], "pbnd": pp.band_np[k],
        })
    return in_maps


def _run(inputs, dims, trace=True):
    N = dims["N"]
    n_loc = -(-N // (NCORES * P * NQ)) * P * NQ  # 12800: divisible by 512
    dims = dict(dims, n_loc=n_loc)
    ep, pp = _plan_all(inputs, dims)
    nc = build_kernel2(dims, ep, pp)
    in_maps = _prep_inputs(inputs, dims, ep, pp)
    res = run_bass_kernel_spmd(nc, in_maps, core_ids=list(range(NCORES)),
                               trace=trace)
    _run.last_exec_time_ns = res.exec_time_ns
    out = np.concatenate([r["out"] for r in res.results], axis=0)[:N]
    return np.ascontiguousarray(out.astype(np.float32))


_run.last_exec_time_ns = None


def kernel(**inputs) -> np.ndarray:
    return _run(inputs, FULL)
